# revision 9
# baseline (speedup 1.0000x reference)
"""Trainium2 Bass kernel for nn_DEPTHVIT_35124242546904 (sparse_attention).

Self-contained: takes FULL inputs, shards batch B=128 over 8 NeuronCores,
returns FULL outputs (out, attn_out, masked).

Algorithm (per core, B_local=16, two groups of 8):
  - q-collapse: attn[b,n] = x[b,n,:] . qk[b,:], qk = q @ Wk  (no big KV matmul)
  - dot products via fused scalar_tensor_tensor on DVE (or POOL mult + ACT
    accumulate), with the forced product output written as fp32r "scratch"
    (= x*qk rounded to 12-bit mantissa; the accumulate is pre-rounding/exact)
  - permutation to chunk order via one-hot matmul (exact in fp32)
  - softmax + lower-median-of-5 (min/max network) + mask on [8,480]
  - xv = sum_n masked*x computed as (sum_n masked*scratch)/qk  (fp32r matmuls)
  - out = LN(xv @ Wv.T @ proj.T + proj_b)
"""
import os
import numpy as np

import concourse.bass as bass
import concourse.mybir as mybir
import concourse.tile as tile
import concourse.bacc as bacc
from concourse.bass_utils import run_bass_kernel_spmd

F32 = mybir.dt.float32
F32R = mybir.dt.float32r
ALU = mybir.AluOpType
AFT = mybir.ActivationFunctionType

C = 768
N = 480
NUM_CHUNKS = 96
CHUNK = 5
Q_SCALE = 0.05103
LN_EPS = 1e-6
PH, PW = 12, 40
B = 128
NCORES = 8
BL = B // NCORES          # 16 batches per core
G = 8                     # group size (softmax granularity)
NGROUPS = BL // G         # 2
NC4 = 4                   # n-chunks of 120
NP = 120                  # partitions per n-chunk
DC = 6                    # d-chunks of 128

# how many of the 8 dots per group go to DVE (rest: POOL mult + ACT accum)
K_DVE = int(os.environ.get("K_DVE", "4"))


def build_kernel():
    nc = bacc.Bacc("TRN2", target_bir_lowering=False, debug=False)

    inp = nc.declare_dram_parameter("input", [BL, N + 1, C], F32, isOutput=False)
    ident_in = nc.declare_dram_parameter("ident", [128, 128], F32, isOutput=False)
    wqT_in = nc.declare_dram_parameter("wqT", [C, C], F32, isOutput=False)
    wk_in = nc.declare_dram_parameter("wk", [C, C], F32, isOutput=False)
    wvT_in = nc.declare_dram_parameter("wvT", [C, C], F32R, isOutput=False)
    projT_in = nc.declare_dram_parameter("projT", [C, C], F32R, isOutput=False)
    projb_in = nc.declare_dram_parameter("projb", [1, C], F32, isOutput=False)
    normw_in = nc.declare_dram_parameter("normw", [1, C], F32, isOutput=False)
    normb_in = nc.declare_dram_parameter("normb", [1, C], F32, isOutput=False)
    p_in = nc.declare_dram_parameter("p_rhs", [N, N], F32, isOutput=False)
    cm_in = nc.declare_dram_parameter("colmask", [G, G], F32, isOutput=False)
    pt_in = nc.declare_dram_parameter("pt_rhs", [N, N], F32R, isOutput=False)

    out_o = nc.declare_dram_parameter("out", [BL, C], F32, isOutput=True)
    ao_o = nc.declare_dram_parameter("attn_out", [BL, N], F32, isOutput=True)
    mk_o = nc.declare_dram_parameter("masked", [BL, N], F32, isOutput=True)

    with tile.TileContext(nc) as tc:
        with (
            tc.tile_pool(name="const", bufs=1) as constp,
            tc.tile_pool(name="dramb", bufs=1, space="DRAM") as dramp,
            tc.tile_pool(name="ps", bufs=1, space="PSUM") as ps,
            tc.tile_pool(name="ps2", bufs=2, space="PSUM") as ps2,
        ):
            qk_d = dramp.tile([BL, C], F32)
            # ---------------- constants (live whole kernel) ----------------
            ident = constp.tile([128, 128], F32)
            nc.sync.dma_start(ident[:], ident_in[:])
            p_rhs = constp.tile([NP, NC4, N], F32)     # [n-chunk part, chunk, i]
            nc.sync.dma_start(
                p_rhs[:], p_in[:].rearrange("(c p) i -> p c i", p=NP))
            pt_rhs = constp.tile([NP, NC4, N], F32R)
            nc.sync.dma_start(
                pt_rhs[:], pt_in[:].rearrange("(c p) i -> p c i", p=NP))
            projb_r = constp.tile([BL, C], F32)
            nc.sync.dma_start(projb_r[:], projb_in[:].broadcast_to([BL, C]))
            normw_r = constp.tile([BL, C], F32)
            nc.sync.dma_start(normw_r[:], normw_in[:].broadcast_to([BL, C]))
            normb_r = constp.tile([BL, C], F32)
            nc.sync.dma_start(normb_r[:], normb_in[:].broadcast_to([BL, C]))
            colm = constp.tile([NP, G, G], F32)
            nc.sync.dma_start(colm[:], cm_in[:].unsqueeze(0).broadcast_to([NP, G, G]))
            eps_t = constp.tile([BL, 1], F32)
            nc.vector.memset(eps_t[:], LN_EPS)
            qk_nat = constp.tile([BL, C], F32)
            xvT = constp.tile([128, DC, BL], F32R)
            rq_g = [constp.tile([G, C], F32, name=f"rq_g{i}") for i in range(NGROUPS)]
            rq = constp.tile([BL, C], F32)

            # ---------------- setup: qT, qkT, qk_nat, rq ----------------
            with tc.tile_pool(name="setup", bufs=1) as sp:
                wqT = sp.tile([128, DC, C], F32)   # wqT[e-chunk part][c]
                nc.sync.dma_start(
                    wqT[:], wqT_in[:].rearrange("(c p) d -> p c d", p=128))
                wk = sp.tile([128, DC, C], F32)    # wk[c-chunk part][d]
                nc.sync.dma_start(
                    wk[:], wk_in[:].rearrange("(c p) d -> p c d", p=128))

                q_in_sb = sp.tile([BL, C], F32)
                nc.sync.dma_start(q_in_sb[:], inp[:, 0, :])

                # q_inT [e, b] via 6 PE transposes
                q_inT = sp.tile([128, DC, BL], F32)
                for j in range(DC):
                    tps = ps.tile([128, BL], F32, tag="sps", name="tps")
                    nc.tensor.transpose(
                        tps[:], q_in_sb[:, 128 * j:128 * (j + 1)],
                        ident[0:BL, 0:BL])
                    nc.scalar.copy(q_inT[:, j, :], tps[:])

                # qT[c,b] = sum_e wqT[e,c] q_inT[e,b]; Q_SCALE on copy-out
                qT = sp.tile([128, DC, BL], F32)
                for m in range(DC):
                    qps = ps.tile([128, BL], F32, tag="sps", name="qps")
                    for e in range(DC):
                        nc.tensor.matmul(
                            qps[:], wqT[:, e, 128 * m:128 * (m + 1)],
                            q_inT[:, e, :],
                            start=(e == 0), stop=(e == DC - 1))
                    nc.scalar.activation(qT[:, m, :], qps[:], AFT.Copy,
                                         scale=Q_SCALE)

                # qkT[d,b] = sum_c wk[c,d] qT[c,b]
                qkT = sp.tile([128, DC, BL], F32)
                for m in range(DC):
                    kps = ps.tile([128, BL], F32, tag="sps", name="kps")
                    for c in range(DC):
                        nc.tensor.matmul(
                            kps[:], wk[:, c, 128 * m:128 * (m + 1)], qT[:, c, :],
                            start=(c == 0), stop=(c == DC - 1))
                    nc.scalar.copy(qkT[:, m, :], kps[:])

                # qk_nat [b, d] via 6 PE transposes of qkT chunks
                qnps = ps2.tile([BL, C], F32, tag="big", name="qnps")
                for j in range(DC):
                    nc.tensor.transpose(
                        qnps[:, 128 * j:128 * (j + 1)], qkT[:, j, :], ident[:])
                nc.scalar.copy(qk_nat[:], qnps[:])
                nc.vector.reciprocal(rq[:], qk_nat[:])
                for i in range(NGROUPS):
                    nc.sync.dma_start(rq_g[i][:], rq[G * i:G * (i + 1), :])
                # DRAM bounce so per-batch rows can be re-loaded at
                # partition 0 for partition_broadcast
                nc.sync.dma_start(qk_d[:], qk_nat[:])

            # ---------------- main per-group pipeline ----------------
            with (
                tc.tile_pool(name="work", bufs=1) as work,
                tc.tile_pool(name="scr", bufs=1) as scrp,
                tc.tile_pool(name="soft", bufs=1) as soft,
            ):
                for g in range(NGROUPS):
                    b0 = g * G
                    attnT = [soft.tile([NP, G], F32, tag=f"attnT{j}",
                                       name=f"attnT{j}") for j in range(NC4)]
                    scr_t = [[scrp.tile([NP, C], F32R, tag=f"scr{gb}_{j}",
                                        name=f"scr{gb}_{j}")
                              for j in range(NC4)] for gb in range(G)]

                    for gb in range(G):
                        b = b0 + gb
                        x_sb = work.tile([NP, NC4, C], F32, tag="x", bufs=2,
                                         name="x_sb")
                        nc.sync.dma_start(
                            x_sb[:],
                            inp[b, 1:, :].rearrange("(c p) d -> p c d", p=NP))
                        qk_row1 = work.tile([1, C], F32, tag="qkr1", bufs=2,
                                            name="qk_row1")
                        nc.sync.dma_start(qk_row1[:], qk_d[b:b + 1, :])
                        qk_rows = work.tile([NP, C], F32, tag="qkr", bufs=2,
                                            name="qk_rows")
                        nc.gpsimd.partition_broadcast(qk_rows[:], qk_row1[:])
                        for j in range(NC4):
                            if gb % G < K_DVE:
                                nc.vector.scalar_tensor_tensor(
                                    out=scr_t[gb][j][:], in0=x_sb[:, j, :],
                                    scalar=1.0, in1=qk_rows[:],
                                    op0=ALU.mult, op1=ALU.mult,
                                    accum_out=attnT[j][:, gb:gb + 1])
                            else:
                                scr32 = work.tile([NP, C], F32, tag="scr32",
                                                  bufs=2, name="scr32")
                                nc.gpsimd.tensor_tensor(
                                    out=scr32[:], in0=x_sb[:, j, :],
                                    in1=qk_rows[:], op=ALU.mult)
                                nc.scalar.activation(
                                    scr_t[gb][j][:], scr32[:], AFT.Copy,
                                    accum_out=attnT[j][:, gb:gb + 1])

                    # part[b,i] = attn[b, perm[i]] (exact fp32 one-hot matmul)
                    part_ps = ps.tile([G, N], F32, tag="pnn", name="part_ps")
                    for j in range(NC4):
                        nc.tensor.matmul(part_ps[:], attnT[j][:], p_rhs[:, j, :],
                                         start=(j == 0), stop=(j == NC4 - 1))

                    # softmax over chunks of 5
                    part_sb = soft.tile([G, N], F32, tag="sA", name="part_sb")
                    nc.scalar.copy(part_sb[:], part_ps[:])
                    pv = part_sb[:].rearrange("p (k j) -> p k j", j=CHUNK)
                    mx = soft.tile([G, NUM_CHUNKS], F32, tag="r0", name="mx")
                    nc.vector.tensor_reduce(out=mx[:], in_=pv,
                                            axis=mybir.AxisListType.X, op=ALU.max)
                    mx5 = soft.tile([G, N], F32, tag="sB", name="mx5")
                    nc.vector.tensor_copy(
                        mx5[:].rearrange("p (k j) -> p k j", j=CHUNK),
                        mx[:].unsqueeze(-1).broadcast_to([G, NUM_CHUNKS, CHUNK]))
                    dif = soft.tile([G, N], F32, tag="sC", name="dif")
                    nc.vector.tensor_tensor(out=dif[:], in0=part_sb[:],
                                            in1=mx5[:], op=ALU.subtract)
                    ex = soft.tile([G, N], F32, tag="sA", name="ex")
                    nc.scalar.activation(ex[:], dif[:], AFT.Exp)
                    sm = soft.tile([G, NUM_CHUNKS], F32, tag="r1", name="sm")
                    nc.vector.tensor_reduce(
                        out=sm[:],
                        in_=ex[:].rearrange("p (k j) -> p k j", j=CHUNK),
                        axis=mybir.AxisListType.X, op=ALU.add)
                    rs = soft.tile([G, NUM_CHUNKS], F32, tag="r2", name="rs")
                    nc.vector.reciprocal(rs[:], sm[:])
                    rs5 = soft.tile([G, N], F32, tag="sB", name="rs5")
                    nc.vector.tensor_copy(
                        rs5[:].rearrange("p (k j) -> p k j", j=CHUNK),
                        rs[:].unsqueeze(-1).broadcast_to([G, NUM_CHUNKS, CHUNK]))
                    p_sm = soft.tile([G, N], F32, tag="sD", name="p_sm")
                    nc.vector.tensor_tensor(out=p_sm[:], in0=ex[:], in1=rs5[:],
                                            op=ALU.mult)

                    # lower-median-of-5 (= true median for odd 5):
                    # med5 = med3(x4, max(min01, min23), min(max01, max23))
                    pj = p_sm[:].rearrange("p (k j) -> p k j", j=CHUNK)
                    xs = [pj[:, :, j] for j in range(CHUNK)]
                    mt = [soft.tile([G, NUM_CHUNKS], F32, tag=f"m{i}",
                                    name=f"mt{i}") for i in range(6)]
                    nc.vector.tensor_tensor(out=mt[0][:], in0=xs[0], in1=xs[1],
                                            op=ALU.min)
                    nc.vector.tensor_tensor(out=mt[1][:], in0=xs[0], in1=xs[1],
                                            op=ALU.max)
                    nc.vector.tensor_tensor(out=mt[2][:], in0=xs[2], in1=xs[3],
                                            op=ALU.min)
                    nc.vector.tensor_tensor(out=mt[3][:], in0=xs[2], in1=xs[3],
                                            op=ALU.max)
                    nc.vector.tensor_tensor(out=mt[4][:], in0=mt[0][:],
                                            in1=mt[2][:], op=ALU.max)
                    nc.vector.tensor_tensor(out=mt[5][:], in0=mt[1][:],
                                            in1=mt[3][:], op=ALU.min)
                    t1 = soft.tile([G, NUM_CHUNKS], F32, tag="m0", name="t1")
                    t2 = soft.tile([G, NUM_CHUNKS], F32, tag="m1", name="t2")
                    t3 = soft.tile([G, NUM_CHUNKS], F32, tag="m2", name="t3")
                    med = soft.tile([G, NUM_CHUNKS], F32, tag="m3", name="med")
                    nc.vector.tensor_tensor(out=t1[:], in0=mt[4][:], in1=mt[5][:],
                                            op=ALU.min)
                    nc.vector.tensor_tensor(out=t2[:], in0=mt[4][:], in1=mt[5][:],
                                            op=ALU.max)
                    nc.vector.tensor_tensor(out=t3[:], in0=t2[:], in1=xs[4],
                                            op=ALU.min)
                    nc.vector.tensor_tensor(out=med[:], in0=t1[:], in1=t3[:],
                                            op=ALU.max)
                    med5 = soft.tile([G, N], F32, tag="sB", name="med5")
                    nc.vector.tensor_copy(
                        med5[:].rearrange("p (k j) -> p k j", j=CHUNK),
                        med[:].unsqueeze(-1).broadcast_to([G, NUM_CHUNKS, CHUNK]))

                    # mask = where(p_sm < med, 0, p_sm) / 48
                    ge = soft.tile([G, N], F32, tag="sC", name="ge")
                    nc.vector.tensor_tensor(out=ge[:], in0=p_sm[:], in1=med5[:],
                                            op=ALU.is_ge)
                    maskS = soft.tile([G, N], F32, tag="sA", name="maskS")
                    nc.vector.scalar_tensor_tensor(
                        out=maskS[:], in0=p_sm[:],
                        scalar=1.0 / (NUM_CHUNKS / 2.0), in1=ge[:],
                        op0=ALU.mult, op1=ALU.mult)

                    # transposes of p_sm / maskS -> [i-part, gb] fp32r chunks
                    p_smT = [soft.tile([NP, G], F32R, tag=f"psmT{j}",
                                       name=f"psmT{j}") for j in range(NC4)]
                    maskT = [soft.tile([NP, G], F32R, tag=f"mskT{j}",
                                       name=f"mskT{j}") for j in range(NC4)]
                    for j in range(NC4):
                        tp1 = ps2.tile([NP, G], F32, tag="tp", name="tp1")
                        nc.tensor.transpose(
                            tp1[:], p_sm[:, NP * j:NP * (j + 1)], ident[0:G, 0:G])
                        nc.scalar.copy(p_smT[j][:], tp1[:])
                        tp2 = ps2.tile([NP, G], F32, tag="tp", name="tp2")
                        nc.tensor.transpose(
                            tp2[:], maskS[:, NP * j:NP * (j + 1)],
                            ident[0:G, 0:G])
                        nc.scalar.copy(maskT[j][:], tp2[:])

                    # un-permute to natural order (fp32r one-hot matmuls)
                    ao_ps = ps.tile([G, N], F32, tag="pnn", name="ao_ps")
                    for j in range(NC4):
                        nc.tensor.matmul(ao_ps[:], p_smT[j][:], pt_rhs[:, j, :],
                                         start=(j == 0), stop=(j == NC4 - 1))
                    ao_sb = soft.tile([G, N], F32, tag="sD2", name="ao_sb")
                    nc.scalar.copy(ao_sb[:], ao_ps[:])
                    nc.sync.dma_start(ao_o[b0:b0 + G, :], ao_sb[:])

                    mn_ps = ps.tile([G, N], F32, tag="pnn", name="mn_ps")
                    for j in range(NC4):
                        nc.tensor.matmul(mn_ps[:], maskT[j][:], pt_rhs[:, j, :],
                                         start=(j == 0), stop=(j == NC4 - 1))
                    mn_sb = soft.tile([G, N], F32, tag="sE", name="mn_sb")
                    nc.scalar.copy(mn_sb[:], mn_ps[:])
                    nc.sync.dma_start(mk_o[b0:b0 + G, :], mn_sb[:])

                    # masked natural -> [n-part, gb] chunks for xv weights
                    mnT = [soft.tile([NP, G], F32, tag=f"mnT{j}",
                                     name=f"mnT{j}") for j in range(NC4)]
                    for j in range(NC4):
                        tp3 = ps2.tile([NP, G], F32, tag="tp", name="tp3")
                        nc.tensor.transpose(
                            tp3[:], mn_sb[:, NP * j:NP * (j + 1)],
                            ident[0:G, 0:G])
                        nc.scalar.copy(mnT[j][:], tp3[:])

                    # xv accumulation: psum_xv[gb, d] = qk[gb,d] * xv[gb,d]
                    xv_ps = ps2.tile([G, 1024], F32, tag="big", name="xv_ps")
                    for gb in range(G):
                        for j in range(NC4):
                            mf = work.tile([NP, G], F32R, tag="mf", bufs=2,
                                           name="mf")
                            nc.vector.tensor_tensor(
                                out=mf[:], in0=mnT[j][:], in1=colm[:, gb, :],
                                op=ALU.mult)
                            first = (gb == 0 and j == 0)
                            last = (gb == G - 1 and j == NC4 - 1)
                            for h in range(2):
                                nc.tensor.matmul(
                                    xv_ps[:, 512 * h:512 * h + 384],
                                    mf[:],
                                    scr_t[gb][j][:, 384 * h:384 * (h + 1)],
                                    start=first, stop=last)
                    # xv = psum_xv * rq  (per-group tile, partitions 0..G)
                    xv_g = soft.tile([G, C], F32, tag="xvg", name="xv_g")
                    nc.vector.tensor_tensor(out=xv_g[:, 0:384],
                                            in0=xv_ps[:, 0:384],
                                            in1=rq_g[g][:, 0:384], op=ALU.mult)
                    nc.vector.tensor_tensor(out=xv_g[:, 384:768],
                                            in0=xv_ps[:, 512:896],
                                            in1=rq_g[g][:, 384:768], op=ALU.mult)
                    # xvT[:, j, b0:b0+G] = xv_g chunks transposed
                    for j in range(DC):
                        tp5 = ps2.tile([128, G], F32, tag="tp", name="tp5")
                        nc.tensor.transpose(
                            tp5[:], xv_g[:, 128 * j:128 * (j + 1)],
                            ident[0:G, 0:G])
                        nc.scalar.copy(xvT[:, j, b0:b0 + G], tp5[:])

            # ---------------- final projection + layernorm (all 16) ----------
            with tc.tile_pool(name="fin", bufs=1) as fp:
                wvT = fp.tile([128, DC, C], F32R)
                nc.sync.dma_start(
                    wvT[:], wvT_in[:].rearrange("(c p) d -> p c d", p=128))
                projT = fp.tile([128, DC, C], F32R)
                nc.sync.dma_start(
                    projT[:], projT_in[:].rearrange("(c p) d -> p c d", p=128))

                # voutT[c,b] = sum_d wvT[d,c] xvT[d,b]
                voutT = fp.tile([128, DC, BL], F32R)
                for m in range(DC):
                    vps = ps.tile([128, BL], F32, tag="sps", name="vps")
                    for d in range(DC):
                        nc.tensor.matmul(
                            vps[:], wvT[:, d, 128 * m:128 * (m + 1)],
                            xvT[:, d, :],
                            start=(d == 0), stop=(d == DC - 1))
                    nc.scalar.copy(voutT[:, m, :], vps[:])

                # outT[c',b] = sum_c projT[c,c'] voutT[c,b]
                outT = fp.tile([128, DC, BL], F32)
                for m in range(DC):
                    ops_ = ps.tile([128, BL], F32, tag="sps", name="ops_")
                    for c in range(DC):
                        nc.tensor.matmul(
                            ops_[:], projT[:, c, 128 * m:128 * (m + 1)],
                            voutT[:, c, :],
                            start=(c == 0), stop=(c == DC - 1))
                    nc.scalar.copy(outT[:, m, :], ops_[:])

                # transpose back to [b, c'] and add bias
                o_nat_ps = ps2.tile([BL, C], F32, tag="big", name="o_nat_ps")
                for j in range(DC):
                    nc.tensor.transpose(
                        o_nat_ps[:, 128 * j:128 * (j + 1)], outT[:, j, :],
                        ident[:])
                o_nat = fp.tile([BL, C], F32)
                nc.vector.tensor_tensor(out=o_nat[:], in0=o_nat_ps[:],
                                        in1=projb_r[:], op=ALU.add)

                # layernorm over c
                ssum = fp.tile([BL, 1], F32)
                nc.vector.tensor_reduce(out=ssum[:], in_=o_nat[:],
                                        axis=mybir.AxisListType.X, op=ALU.add)
                mu = fp.tile([BL, 1], F32)
                nc.vector.tensor_scalar(out=mu[:], in0=ssum[:], scalar1=1.0 / C,
                                        scalar2=None, op0=ALU.mult)
                ctr = fp.tile([BL, C], F32)
                nc.vector.tensor_scalar(out=ctr[:], in0=o_nat[:], scalar1=mu[:],
                                        scalar2=None, op0=ALU.subtract)
                sqd = fp.tile([BL, C], F32)
                ssq = fp.tile([BL, 1], F32)
                nc.vector.scalar_tensor_tensor(out=sqd[:], in0=ctr[:],
                                               scalar=1.0, in1=ctr[:],
                                               op0=ALU.mult, op1=ALU.mult,
                                               accum_out=ssq[:])
                var = fp.tile([BL, 1], F32)
                nc.vector.tensor_scalar(out=var[:], in0=ssq[:], scalar1=1.0 / C,
                                        scalar2=None, op0=ALU.mult)
                sd = fp.tile([BL, 1], F32)
                nc.scalar.activation(sd[:], var[:], AFT.Sqrt, bias=eps_t[:])
                rsd = fp.tile([BL, 1], F32)
                nc.vector.reciprocal(rsd[:], sd[:])
                nrm = fp.tile([BL, C], F32)
                nc.vector.scalar_tensor_tensor(out=nrm[:], in0=ctr[:],
                                               scalar=rsd[:], in1=normw_r[:],
                                               op0=ALU.mult, op1=ALU.mult)
                o_fin = fp.tile([BL, C], F32)
                nc.vector.tensor_tensor(out=o_fin[:], in0=nrm[:], in1=normb_r[:],
                                        op=ALU.add)
                nc.sync.dma_start(out_o[:], o_fin[:])

    nc.compile()
    return nc


_NC_CACHE = None


def kernel(input, perm, to_kv_w, to_q_w, proj_w, proj_b, norm_w, norm_b):
    global _NC_CACHE
    input = np.ascontiguousarray(np.asarray(input, dtype=np.float32))
    perm = np.asarray(perm, dtype=np.int32)

    if _NC_CACHE is None:
        _NC_CACHE = build_kernel()
    nc = _NC_CACHE

    p_rhs = np.zeros((N, N), dtype=np.float32)   # p_rhs[n, i] = 1 iff perm[i]==n
    p_rhs[perm, np.arange(N)] = 1.0
    pt_rhs = np.ascontiguousarray(p_rhs.T)       # pt_rhs[i, n] = 1 iff perm[i]==n

    shared = {
        "ident": np.eye(128, dtype=np.float32),
        "wqT": np.ascontiguousarray(np.asarray(to_q_w, np.float32).T),
        "wk": np.ascontiguousarray(np.asarray(to_kv_w, np.float32)[:C]),
        "wvT": np.ascontiguousarray(np.asarray(to_kv_w, np.float32)[C:].T),
        "projT": np.ascontiguousarray(np.asarray(proj_w, np.float32).T),
        "projb": np.asarray(proj_b, np.float32).reshape(1, C),
        "normw": np.asarray(norm_w, np.float32).reshape(1, C),
        "normb": np.asarray(norm_b, np.float32).reshape(1, C),
        "p_rhs": p_rhs,
        "colmask": np.eye(G, dtype=np.float32),
        "pt_rhs": pt_rhs,
    }
    in_maps = [
        {"input": input[BL * i:BL * (i + 1)], **shared} for i in range(NCORES)
    ]
    res = run_bass_kernel_spmd(nc, in_maps, list(range(NCORES)))
    out = np.concatenate([r["out"] for r in res.results], axis=0)[:, None, :]
    ao = np.concatenate([r["attn_out"] for r in res.results], axis=0)
    mk = np.concatenate([r["masked"] for r in res.results], axis=0)
    return out, ao.reshape(B, PH, PW), mk.reshape(B, PH, PW)
